# revision 4
# baseline (speedup 1.0000x reference)
"""MoE gate (softmax + top-2 + load-balance loss) on 8 Trainium2 NeuronCores.

Data-parallel: token dim (16384) sharded 8 ways, gate weight replicated.
Per core: logits = x_shard @ W.T via PE (f32), softmax + top-2 via DVE/ACT,
partial per-expert prob sums returned per core; the scalar load-balance loss
is finished on host (tiny reduction).

x arrives [n, d] but the PE contracts along the partition axis, so each
core's shard is fed pre-transposed ([d, n], done on host) — natural-layout
DMA loads, no on-device transpose.
"""

import numpy as np

import concourse.bacc as bacc
import concourse.mybir as mybir
import concourse.tile as tile
from concourse import bass_utils

N_TOKENS = 16384
D_MODEL = 4096
NUM_EXPERTS = 64
TOP_K = 2
EPS = 1e-8
N_CORES = 8

TPC = N_TOKENS // N_CORES  # tokens per core (2048)
P = 128                    # partitions
DC = D_MODEL // P          # contraction chunks (32)
TB = 512                   # token block (moving-free-dim max for one SBUF x tile)
NB = TPC // TB             # token blocks per core (4)
JT = TB // P               # 128-token tiles per block (4)

_PROGRAM = None
LAST_RESULT = None  # BassKernelResults of the most recent run (for test harness)


def _build_program():
    nc = bacc.Bacc(
        "TRN2",
        target_bir_lowering=False,
        debug=False,
        enable_asserts=False,
        num_devices=N_CORES,
    )
    f32 = mybir.dt.float32
    xT = nc.dram_tensor("xT", [D_MODEL, TPC], f32, kind="ExternalInput")
    wT = nc.dram_tensor("wT", [D_MODEL, NUM_EXPERTS], f32, kind="ExternalInput")
    scores = nc.dram_tensor("scores", [TPC, TOP_K], f32, kind="ExternalOutput")
    idx = nc.dram_tensor("idx", [TPC, TOP_K], mybir.dt.uint32, kind="ExternalOutput")
    pacc = nc.dram_tensor("pacc", [P, NUM_EXPERTS], f32, kind="ExternalOutput")

    E = NUM_EXPERTS
    with tile.TileContext(nc) as tc:
        with (
            tc.tile_pool(name="wp", bufs=1) as wpool,
            tc.tile_pool(name="xp", bufs=6) as xpool,
            tc.tile_pool(name="pp", bufs=8, space="PSUM") as pspool,
            tc.tile_pool(name="sp", bufs=3) as spool,
            tc.tile_pool(name="ap", bufs=1) as apool,
        ):
            # Gate weight, all 32 K-chunks resident: chunk d at wt_sb[:, d, :]
            wt_sb = wpool.tile([P, DC, E], f32)
            nc.sync.dma_start(wt_sb[:], wT.ap().rearrange("(c p) e -> p c e", p=P))

            acc_sb = apool.tile([P, E], f32)
            nc.vector.memset(acc_sb[:], 0.0)

            for tb in range(NB):
                psums = [
                    pspool.tile([P, E], f32, tag="psum", name=f"psum{tb}_{j}")
                    for j in range(JT)
                ]
                for d in range(DC):
                    xt = xpool.tile([P, TB], f32)
                    nc.sync.dma_start(
                        xt[:], xT.ap()[d * P:(d + 1) * P, tb * TB:(tb + 1) * TB]
                    )
                    for j in range(JT):
                        # logits[n,e] += x[n,dchunk] @ W[e,dchunk].T
                        nc.tensor.matmul(
                            psums[j][:],
                            xt[:, j * P:(j + 1) * P],   # stationary [K=d, M=n]
                            wt_sb[:, d, :],             # moving     [K=d, N=e]
                            start=(d == 0),
                            stop=(d == DC - 1),
                        )
                for j in range(JT):
                    tok0 = tb * TB + j * P
                    logits = psums[j]
                    neg_m = spool.tile([P, 1], f32, tag="negm")
                    nc.vector.reduce_max(
                        neg_m[:], logits[:], axis=mybir.AxisListType.X, negate=True
                    )
                    e_sb = spool.tile([P, E], f32, tag="esb")
                    s = spool.tile([P, 1], f32, tag="s")
                    nc.scalar.activation(
                        e_sb[:], logits[:], mybir.ActivationFunctionType.Exp,
                        bias=neg_m[:], accum_out=s[:],
                    )
                    r = spool.tile([P, 1], f32, tag="r")
                    nc.vector.reciprocal(r[:], s[:])
                    probs = spool.tile([P, E], f32, tag="probs")
                    nc.vector.tensor_scalar_mul(probs[:], e_sb[:], r[:])
                    nc.vector.tensor_add(acc_sb[:], acc_sb[:], probs[:])

                    # top-2 on exp values (same order as probs; norm cancels)
                    max8 = spool.tile([P, 8], f32, tag="max8")
                    nc.vector.max(out=max8[:], in_=e_sb[:])
                    idx8 = spool.tile([P, 8], mybir.dt.uint32, tag="idx8")
                    nc.vector.max_index(idx8[:], max8[:], e_sb[:])
                    s2 = spool.tile([P, 1], f32, tag="s2")
                    nc.vector.reduce_sum(
                        s2[:], max8[:, 0:TOP_K], axis=mybir.AxisListType.X
                    )
                    r2 = spool.tile([P, 1], f32, tag="r2")
                    nc.vector.reciprocal(r2[:], s2[:])
                    sc = spool.tile([P, TOP_K], f32, tag="sc")
                    nc.vector.tensor_scalar_mul(sc[:], max8[:, 0:TOP_K], r2[:])

                    nc.sync.dma_start(scores.ap()[tok0:tok0 + P, :], sc[:])
                    nc.sync.dma_start(idx.ap()[tok0:tok0 + P, :], idx8[:, 0:TOP_K])

            nc.sync.dma_start(pacc.ap()[:], acc_sb[:])

    nc.compile()
    return nc


def get_program():
    global _PROGRAM
    if _PROGRAM is None:
        _PROGRAM = _build_program()
    return _PROGRAM


def kernel(x: np.ndarray, W: np.ndarray):
    global LAST_RESULT
    nc = get_program()
    x = np.asarray(x, dtype=np.float32)
    W = np.asarray(W, dtype=np.float32)
    wT = np.ascontiguousarray(W.T)
    in_maps = [
        {
            "xT": np.ascontiguousarray(x[i * TPC:(i + 1) * TPC, :].T),
            "wT": wT,
        }
        for i in range(N_CORES)
    ]
    res = bass_utils.run_bass_kernel_spmd(nc, in_maps, core_ids=list(range(N_CORES)))
    LAST_RESULT = res
    outs = res.results

    topk_scores = np.concatenate([np.asarray(o["scores"]) for o in outs], axis=0)
    topk_indices = (
        np.concatenate([np.asarray(o["idx"]) for o in outs], axis=0)
        .astype(np.uint32)
        .view(np.int32)
    )
    part = np.stack([np.asarray(o["pacc"]) for o in outs], axis=0)  # [8, 128, E]
    expert_prob = part.astype(np.float64).sum(axis=(0, 1)) / N_TOKENS
    loss = np.float32(np.sum(expert_prob * np.log(expert_prob + EPS)))
    return topk_scores, topk_indices, loss


if __name__ == "__main__":
    rng = np.random.default_rng(0)
    x = rng.standard_normal((N_TOKENS, D_MODEL), dtype=np.float32)
    W = rng.standard_normal((NUM_EXPERTS, D_MODEL), dtype=np.float32) / np.sqrt(D_MODEL)
    s, i, l = kernel(x, W)
    print(s.shape, s.dtype, i.shape, i.dtype, l)


# revision 5
# speedup vs baseline: 1.3258x; 1.3258x over previous
"""MoE gate (softmax + top-2 + load-balance loss) on 8 Trainium2 NeuronCores.

Data-parallel: token dim (16384) sharded 8 ways, gate weight replicated.
Per core: logits = x_shard @ W.T via PE (f32), softmax + top-2 via DVE/ACT,
partial per-expert prob sums returned per core; the scalar load-balance loss
is finished on host (tiny reduction).

x arrives [n, d] but the PE contracts along the partition axis, so each
core's shard is fed pre-transposed ([d, n], done on host) — natural-layout
DMA loads, no on-device transpose.

Matmul arrangement: the small gate weight W^T chunk [128, 64] is the
stationary operand (cheap reload), x streams as the moving operand with the
full 512-wide free dim, producing logits transposed [64 experts, 512 tokens]
in PSUM. A PE transpose (identity matmul) flips each 128-token slice back to
[128, 64] for the row-wise softmax/top-k chain.
"""

import os

import numpy as np

import concourse.bacc as bacc
import concourse.mybir as mybir
import concourse.tile as tile
from concourse import bass_utils
from concourse.masks import make_identity

N_TOKENS = 16384
D_MODEL = 4096
NUM_EXPERTS = 64
TOP_K = 2
EPS = 1e-8
N_CORES = 8

TPC = N_TOKENS // N_CORES  # tokens per core (2048)
P = 128                    # partitions
DC = D_MODEL // P          # contraction chunks (32)
TB = 512                   # token block (fp32 moving-free-dim / PSUM bank max)
NB = TPC // TB             # token blocks per core (4)
JT = TB // P               # 128-token tiles per block (4)

_PROGRAM = None
LAST_RESULT = None  # BassKernelResults of the most recent run (for test harness)


def _build_program():
    use_f32r = bool(int(os.environ.get("MOE_F32R", "0")))
    nc = bacc.Bacc(
        "TRN2",
        target_bir_lowering=False,
        debug=False,
        enable_asserts=False,
        num_devices=N_CORES,
    )
    f32 = mybir.dt.float32
    mm_dt = mybir.dt.float32r if use_f32r else f32
    xT = nc.dram_tensor("xT", [D_MODEL, TPC], f32, kind="ExternalInput")
    wT = nc.dram_tensor("wT", [D_MODEL, NUM_EXPERTS], f32, kind="ExternalInput")
    scores = nc.dram_tensor("scores", [TPC, TOP_K], f32, kind="ExternalOutput")
    idx = nc.dram_tensor("idx", [TPC, TOP_K], mybir.dt.uint32, kind="ExternalOutput")
    pacc = nc.dram_tensor("pacc", [P, NUM_EXPERTS], f32, kind="ExternalOutput")

    E = NUM_EXPERTS
    with tile.TileContext(nc) as tc:
        with (
            tc.tile_pool(name="wp", bufs=1) as wpool,
            tc.tile_pool(name="xp", bufs=6) as xpool,
            tc.tile_pool(name="pp", bufs=2, space="PSUM") as pspool,
            tc.tile_pool(name="sp", bufs=3) as spool,
            tc.tile_pool(name="ap", bufs=1) as apool,
        ):
            # Gate weight, all 32 K-chunks resident: chunk d at wt_sb[:, d, :]
            wt_sb = wpool.tile([P, DC, E], f32)
            nc.sync.dma_start(wt_sb[:], wT.ap().rearrange("(c p) e -> p c e", p=P))
            ident = wpool.tile([E, E], f32)
            make_identity(nc, ident[:])

            acc_sb = apool.tile([P, E], f32)
            nc.vector.memset(acc_sb[:], 0.0)

            for tb in range(NB):
                # logits^T for this 512-token block: [experts, tokens]
                psumT = pspool.tile([E, TB], f32, tag="pT", name=f"pT{tb}", bufs=2)
                for d in range(DC):
                    xt = xpool.tile([P, TB], f32, tag="xt", name=f"xt{tb}_{d}")
                    nc.sync.dma_start(
                        xt[:], xT.ap()[d * P:(d + 1) * P, tb * TB:(tb + 1) * TB]
                    )
                    nc.tensor.matmul(
                        psumT[:],
                        wt_sb[:, d, :].bitcast(mm_dt),  # stationary [K=d, M=e]
                        xt[:].bitcast(mm_dt),           # moving     [K=d, N=n]
                        start=(d == 0),
                        stop=(d == DC - 1),
                    )
                lt = spool.tile([E, TB], f32, tag="lt")
                nc.vector.tensor_copy(lt[:], psumT[:])
                psums = [
                    pspool.tile([P, E], f32, tag="psum", name=f"psum{tb}_{j}", bufs=4)
                    for j in range(JT)
                ]
                for j in range(JT):
                    nc.tensor.transpose(
                        psums[j][:], lt[:, j * P:(j + 1) * P], ident[:]
                    )
                for j in range(JT):
                    tok0 = tb * TB + j * P
                    logits = psums[j]
                    neg_m = spool.tile([P, 1], f32, tag="negm")
                    nc.vector.reduce_max(
                        neg_m[:], logits[:], axis=mybir.AxisListType.X, negate=True
                    )
                    e_sb = spool.tile([P, E], f32, tag="esb")
                    s = spool.tile([P, 1], f32, tag="s")
                    nc.scalar.activation(
                        e_sb[:], logits[:], mybir.ActivationFunctionType.Exp,
                        bias=neg_m[:], accum_out=s[:],
                    )
                    r = spool.tile([P, 1], f32, tag="r")
                    nc.vector.reciprocal(r[:], s[:])
                    probs = spool.tile([P, E], f32, tag="probs")
                    nc.vector.tensor_scalar_mul(probs[:], e_sb[:], r[:])
                    nc.vector.tensor_add(acc_sb[:], acc_sb[:], probs[:])

                    # top-2 on exp values (same order as probs; norm cancels)
                    max8 = spool.tile([P, 8], f32, tag="max8")
                    nc.vector.max(out=max8[:], in_=e_sb[:])
                    idx8 = spool.tile([P, 8], mybir.dt.uint32, tag="idx8")
                    nc.vector.max_index(idx8[:], max8[:], e_sb[:])
                    s2 = spool.tile([P, 1], f32, tag="s2")
                    nc.vector.reduce_sum(
                        s2[:], max8[:, 0:TOP_K], axis=mybir.AxisListType.X
                    )
                    r2 = spool.tile([P, 1], f32, tag="r2")
                    nc.vector.reciprocal(r2[:], s2[:])
                    sc = spool.tile([P, TOP_K], f32, tag="sc")
                    nc.vector.tensor_scalar_mul(sc[:], max8[:, 0:TOP_K], r2[:])

                    nc.sync.dma_start(scores.ap()[tok0:tok0 + P, :], sc[:])
                    nc.sync.dma_start(idx.ap()[tok0:tok0 + P, :], idx8[:, 0:TOP_K])

            nc.sync.dma_start(pacc.ap()[:], acc_sb[:])

    nc.compile()
    return nc


def get_program():
    global _PROGRAM
    if _PROGRAM is None:
        _PROGRAM = _build_program()
    return _PROGRAM


def kernel(x: np.ndarray, W: np.ndarray):
    global LAST_RESULT
    nc = get_program()
    x = np.asarray(x, dtype=np.float32)
    W = np.asarray(W, dtype=np.float32)
    wT = np.ascontiguousarray(W.T)
    in_maps = [
        {
            "xT": np.ascontiguousarray(x[i * TPC:(i + 1) * TPC, :].T),
            "wT": wT,
        }
        for i in range(N_CORES)
    ]
    res = bass_utils.run_bass_kernel_spmd(nc, in_maps, core_ids=list(range(N_CORES)))
    LAST_RESULT = res
    outs = res.results

    topk_scores = np.concatenate([np.asarray(o["scores"]) for o in outs], axis=0)
    topk_indices = (
        np.concatenate([np.asarray(o["idx"]) for o in outs], axis=0)
        .astype(np.uint32)
        .view(np.int32)
    )
    part = np.stack([np.asarray(o["pacc"]) for o in outs], axis=0)  # [8, 128, E]
    expert_prob = part.astype(np.float64).sum(axis=(0, 1)) / N_TOKENS
    loss = np.float32(np.sum(expert_prob * np.log(expert_prob + EPS)))
    return topk_scores, topk_indices, loss


if __name__ == "__main__":
    rng = np.random.default_rng(0)
    x = rng.standard_normal((N_TOKENS, D_MODEL), dtype=np.float32)
    W = rng.standard_normal((NUM_EXPERTS, D_MODEL), dtype=np.float32) / np.sqrt(D_MODEL)
    s, i, l = kernel(x, W)
    print(s.shape, s.dtype, i.shape, i.dtype, l)


# revision 8
# speedup vs baseline: 1.4798x; 1.1162x over previous
"""MoE gate (softmax + top-2 + load-balance loss) on 8 Trainium2 NeuronCores.

Data-parallel: token dim (16384) sharded 8 ways, gate weight replicated.
Per core: logits = x_shard @ W.T via PE (f32), softmax + top-2 via DVE/ACT,
partial per-expert prob sums returned per core; the scalar load-balance loss
is finished on host (tiny reduction).

x arrives [n, d] but the PE contracts along the partition axis, so each
core's shard is fed pre-transposed ([d, n], done on host) — natural-layout
DMA loads, no on-device transpose.

Matmul arrangement: the small gate weight W^T chunk [128, 64] is the
stationary operand (cheap reload), x streams as the moving operand with the
full 512-wide free dim, producing logits transposed [64 experts, 512 tokens]
in PSUM. A PE transpose (identity matmul) flips each 128-token slice back to
[128, 64] for the row-wise softmax/top-k chain.
"""

import os

import numpy as np

import concourse.bacc as bacc
import concourse.mybir as mybir
import concourse.tile as tile
from concourse import bass_utils
from concourse.masks import make_identity

N_TOKENS = 16384
D_MODEL = 4096
NUM_EXPERTS = 64
TOP_K = 2
EPS = 1e-8
N_CORES = 8

TPC = N_TOKENS // N_CORES  # tokens per core (2048)
P = 128                    # partitions
DC = D_MODEL // P          # contraction chunks (32)
TB = 512                   # token block (fp32 moving-free-dim / PSUM bank max)
NB = TPC // TB             # token blocks per core (4)
JT = TB // P               # 128-token tiles per block (4)

_PROGRAM = None
LAST_RESULT = None  # BassKernelResults of the most recent run (for test harness)


def _build_program():
    use_f32r = bool(int(os.environ.get("MOE_F32R", "0")))
    nc = bacc.Bacc(
        "TRN2",
        target_bir_lowering=False,
        debug=False,
        enable_asserts=False,
        num_devices=N_CORES,
    )
    f32 = mybir.dt.float32
    mm_dt = mybir.dt.float32r if use_f32r else f32
    xT = nc.dram_tensor("xT", [D_MODEL, TPC], f32, kind="ExternalInput")
    wT = nc.dram_tensor("wT", [D_MODEL, NUM_EXPERTS], f32, kind="ExternalInput")
    scores = nc.dram_tensor("scores", [TPC, TOP_K], f32, kind="ExternalOutput")
    idx = nc.dram_tensor("idx", [TPC, TOP_K], mybir.dt.uint32, kind="ExternalOutput")
    pacc = nc.dram_tensor("pacc", [P, NUM_EXPERTS], f32, kind="ExternalOutput")

    E = NUM_EXPERTS
    with tile.TileContext(nc) as tc:
        with (
            tc.tile_pool(name="wp", bufs=1) as wpool,
            tc.tile_pool(name="xp", bufs=6) as xpool,
            tc.tile_pool(name="pp", bufs=2, space="PSUM") as pspool,
            tc.tile_pool(name="sp", bufs=3) as spool,
            tc.tile_pool(name="ap", bufs=1) as apool,
        ):
            # Gate weight, all 32 K-chunks resident: chunk d at wt_sb[:, d, :]
            wt_sb = wpool.tile([P, DC, E], f32)
            nc.sync.dma_start(wt_sb[:], wT.ap().rearrange("(c p) e -> p c e", p=P))
            ident = wpool.tile([E, E], f32)
            make_identity(nc, ident[:])

            acc_sb = apool.tile([P, E], f32)
            nc.vector.memset(acc_sb[:], 0.0)

            for tb in range(NB):
                # logits^T for this 512-token block, two half-sums stacked on
                # the partition axis: [0:64] = even d-chunks, [64:128] = odd.
                # The two col-groups of the PE array run concurrently
                # (tile_position), using all 128 columns despite M=64.
                psumT = pspool.tile([P, TB], f32, tag="pT", name=f"pT{tb}", bufs=2)
                for dp in range(DC // 2):
                    xt = xpool.tile([P, 2, TB], f32, tag="xt", name=f"xt{tb}_{dp}")
                    nc.sync.dma_start(
                        xt[:],
                        xT.ap()[2 * dp * P:(2 * dp + 2) * P,
                                tb * TB:(tb + 1) * TB]
                        .rearrange("(c p) n -> p c n", c=2),
                    )
                    for c in range(2):
                        nc.tensor.matmul(
                            psumT[c * E:(c + 1) * E, :],
                            wt_sb[:, 2 * dp + c, :].bitcast(mm_dt),  # [K=d, M=e]
                            xt[:, c, :].bitcast(mm_dt),              # [K=d, N=n]
                            start=(dp == 0),
                            stop=(dp == DC // 2 - 1),
                            tile_position=(0, c * E),
                            skip_group_check=True,
                        )
                lt = spool.tile([E, TB], f32, tag="lt")
                nc.vector.tensor_copy(lt[:], psumT[0:E, :])
                nc.vector.tensor_add(lt[:], lt[:], psumT[E:2 * E, :])
                psums = [
                    pspool.tile([P, E], f32, tag="psum", name=f"psum{tb}_{j}", bufs=4)
                    for j in range(JT)
                ]
                for j in range(JT):
                    nc.tensor.transpose(
                        psums[j][:], lt[:, j * P:(j + 1) * P], ident[:]
                    )
                for j in range(JT):
                    tok0 = tb * TB + j * P
                    logits = psums[j]
                    neg_m = spool.tile([P, 1], f32, tag="negm")
                    nc.vector.reduce_max(
                        neg_m[:], logits[:], axis=mybir.AxisListType.X, negate=True
                    )
                    e_sb = spool.tile([P, E], f32, tag="esb")
                    s = spool.tile([P, 1], f32, tag="s")
                    nc.scalar.activation(
                        e_sb[:], logits[:], mybir.ActivationFunctionType.Exp,
                        bias=neg_m[:], accum_out=s[:],
                    )
                    r = spool.tile([P, 1], f32, tag="r")
                    nc.vector.reciprocal(r[:], s[:])
                    probs = spool.tile([P, E], f32, tag="probs")
                    nc.vector.tensor_scalar_mul(probs[:], e_sb[:], r[:])
                    nc.vector.tensor_add(acc_sb[:], acc_sb[:], probs[:])

                    # top-2 on exp values (same order as probs; norm cancels)
                    max8 = spool.tile([P, 8], f32, tag="max8")
                    nc.vector.max(out=max8[:], in_=e_sb[:])
                    idx8 = spool.tile([P, 8], mybir.dt.uint32, tag="idx8")
                    nc.vector.max_index(idx8[:], max8[:], e_sb[:])
                    s2 = spool.tile([P, 1], f32, tag="s2")
                    nc.vector.reduce_sum(
                        s2[:], max8[:, 0:TOP_K], axis=mybir.AxisListType.X
                    )
                    r2 = spool.tile([P, 1], f32, tag="r2")
                    nc.vector.reciprocal(r2[:], s2[:])
                    sc = spool.tile([P, TOP_K], f32, tag="sc")
                    nc.vector.tensor_scalar_mul(sc[:], max8[:, 0:TOP_K], r2[:])

                    nc.sync.dma_start(scores.ap()[tok0:tok0 + P, :], sc[:])
                    nc.sync.dma_start(idx.ap()[tok0:tok0 + P, :], idx8[:, 0:TOP_K])

            nc.sync.dma_start(pacc.ap()[:], acc_sb[:])

    nc.compile()
    return nc


def get_program():
    global _PROGRAM
    if _PROGRAM is None:
        _PROGRAM = _build_program()
    return _PROGRAM


def kernel(x: np.ndarray, W: np.ndarray):
    global LAST_RESULT
    nc = get_program()
    x = np.asarray(x, dtype=np.float32)
    W = np.asarray(W, dtype=np.float32)
    wT = np.ascontiguousarray(W.T)
    in_maps = [
        {
            "xT": np.ascontiguousarray(x[i * TPC:(i + 1) * TPC, :].T),
            "wT": wT,
        }
        for i in range(N_CORES)
    ]
    res = bass_utils.run_bass_kernel_spmd(nc, in_maps, core_ids=list(range(N_CORES)))
    LAST_RESULT = res
    outs = res.results

    topk_scores = np.concatenate([np.asarray(o["scores"]) for o in outs], axis=0)
    topk_indices = (
        np.concatenate([np.asarray(o["idx"]) for o in outs], axis=0)
        .astype(np.uint32)
        .view(np.int32)
    )
    part = np.stack([np.asarray(o["pacc"]) for o in outs], axis=0)  # [8, 128, E]
    expert_prob = part.astype(np.float64).sum(axis=(0, 1)) / N_TOKENS
    loss = np.float32(np.sum(expert_prob * np.log(expert_prob + EPS)))
    return topk_scores, topk_indices, loss


if __name__ == "__main__":
    rng = np.random.default_rng(0)
    x = rng.standard_normal((N_TOKENS, D_MODEL), dtype=np.float32)
    W = rng.standard_normal((NUM_EXPERTS, D_MODEL), dtype=np.float32) / np.sqrt(D_MODEL)
    s, i, l = kernel(x, W)
    print(s.shape, s.dtype, i.shape, i.dtype, l)


# revision 9
# speedup vs baseline: 1.8244x; 1.2328x over previous
"""MoE gate (softmax + top-2 + load-balance loss) on 8 Trainium2 NeuronCores.

Data-parallel: token dim (16384) sharded 8 ways, gate weight replicated.
Per core: logits = x_shard @ W.T via PE (f32), softmax + top-2 via DVE/ACT,
partial per-expert prob sums returned per core; the scalar load-balance loss
is finished on host (tiny reduction).

x arrives [n, d] but the PE contracts along the partition axis, so each
core's shard is fed pre-transposed ([d, n], done on host) — natural-layout
DMA loads, no on-device transpose.

Matmul arrangement: the small gate weight W^T chunk [128, 64] is the
stationary operand (cheap reload), x streams as the moving operand with the
full 512-wide free dim, producing logits transposed [64 experts, 512 tokens]
in PSUM. A PE transpose (identity matmul) flips each 128-token slice back to
[128, 64] for the row-wise softmax/top-k chain.
"""

import os

import numpy as np

import concourse.bacc as bacc
import concourse.mybir as mybir
import concourse.tile as tile
from concourse import bass_utils
from concourse.masks import make_identity

N_TOKENS = 16384
D_MODEL = 4096
NUM_EXPERTS = 64
TOP_K = 2
EPS = 1e-8
N_CORES = 8

TPC = N_TOKENS // N_CORES  # tokens per core (2048)
P = 128                    # partitions
DC = D_MODEL // P          # contraction chunks (32)
TB = 512                   # token block (fp32 moving-free-dim / PSUM bank max)
NB = TPC // TB             # token blocks per core (4)
JT = TB // P               # 128-token tiles per block (4)

_PROGRAM = None
LAST_RESULT = None  # BassKernelResults of the most recent run (for test harness)


def _build_program():
    use_f32r = bool(int(os.environ.get("MOE_F32R", "0")))
    nc = bacc.Bacc(
        "TRN2",
        target_bir_lowering=False,
        debug=False,
        enable_asserts=False,
        num_devices=N_CORES,
    )
    f32 = mybir.dt.float32
    mm_dt = mybir.dt.float32r if use_f32r else f32
    xT = nc.dram_tensor("xT", [D_MODEL, TPC], f32, kind="ExternalInput")
    wT = nc.dram_tensor("wT", [D_MODEL, NUM_EXPERTS], f32, kind="ExternalInput")
    scores = nc.dram_tensor("scores", [TPC, TOP_K], f32, kind="ExternalOutput")
    idx = nc.dram_tensor("idx", [TPC, TOP_K], mybir.dt.uint32, kind="ExternalOutput")
    pacc = nc.dram_tensor("pacc", [P, NUM_EXPERTS], f32, kind="ExternalOutput")

    E = NUM_EXPERTS
    with tile.TileContext(nc) as tc:
        with (
            tc.tile_pool(name="wp", bufs=1) as wpool,
            tc.tile_pool(name="xp", bufs=6) as xpool,
            tc.tile_pool(name="pp", bufs=2, space="PSUM") as pspool,
            tc.tile_pool(name="sp", bufs=3) as spool,
            tc.tile_pool(name="ap", bufs=1) as apool,
        ):
            # Gate weight, all 32 K-chunks resident: chunk d at wt_sb[:, d, :]
            wt_sb = wpool.tile([P, DC, E], f32)
            nc.sync.dma_start(wt_sb[:], wT.ap().rearrange("(c p) e -> p c e", p=P))
            ident = wpool.tile([E, E], f32)
            make_identity(nc, ident[:])

            acc_sb = apool.tile([P, E], f32)
            nc.vector.memset(acc_sb[:], 0.0)

            X = mybir.AxisListType.X
            for h in range(2):  # halves of this core's tokens (1024 each)
                # logits^T per 512-token sub-block, two half-sums stacked on
                # the partition axis: [0:64] = even d-chunks, [64:128] = odd.
                # The two col-groups of the PE array run concurrently
                # (tile_position), using all 128 columns despite M=64.
                pT = [
                    pspool.tile([P, TB], f32, tag="pT", name=f"pT{h}_{b}", bufs=4)
                    for b in range(2)
                ]
                for dp in range(DC // 2):
                    xt = xpool.tile([P, 2, 2 * TB], f32, tag="xt",
                                    name=f"xt{h}_{dp}", bufs=8)
                    nc.sync.dma_start(
                        xt[:],
                        xT.ap()[2 * dp * P:(2 * dp + 2) * P,
                                h * 2 * TB:(h + 1) * 2 * TB]
                        .rearrange("(c p) n -> p c n", c=2),
                    )
                    for b in range(2):
                        for c in range(2):
                            nc.tensor.matmul(
                                pT[b][c * E:(c + 1) * E, :],
                                wt_sb[:, 2 * dp + c, :].bitcast(mm_dt),
                                xt[:, c, b * TB:(b + 1) * TB].bitcast(mm_dt),
                                start=(dp == 0),
                                stop=(dp == DC // 2 - 1),
                                tile_position=(0, c * E),
                                skip_group_check=True,
                            )
                for b in range(2):
                    tb = 2 * h + b
                    lt = spool.tile([E, TB], f32, tag="lt")
                    nc.vector.tensor_copy(lt[:], pT[b][0:E, :])
                    nc.vector.tensor_add(lt[:], lt[:], pT[b][E:2 * E, :])
                    # [128 tokens, 4 tiles, 64 experts] logits, one PSUM bank
                    p4 = pspool.tile([P, JT, E], f32, tag="p4",
                                     name=f"p4_{tb}", bufs=2)
                    for j in range(JT):
                        nc.tensor.transpose(
                            p4[:, j, :], lt[:, j * P:(j + 1) * P], ident[:]
                        )
                    neg_m = spool.tile([P, JT], f32, tag="negm")
                    nc.vector.reduce_max(neg_m[:], p4[:], axis=X, negate=True)
                    epre = spool.tile([P, JT, E], f32, tag="epre")
                    nc.vector.tensor_add(
                        epre[:], p4[:], neg_m[:].to_broadcast([P, JT, E])
                    )
                    ex = spool.tile([P, JT, E], f32, tag="ex")
                    nc.scalar.activation(
                        ex[:], epre[:], mybir.ActivationFunctionType.Exp
                    )
                    s = spool.tile([P, JT], f32, tag="s")
                    nc.vector.reduce_sum(s[:], ex[:], axis=X)
                    r = spool.tile([P, JT], f32, tag="r")
                    nc.vector.reciprocal(r[:], s[:])
                    probs = spool.tile([P, JT, E], f32, tag="probs")
                    nc.vector.tensor_mul(
                        probs[:], ex[:], r[:].to_broadcast([P, JT, E])
                    )
                    gsum = spool.tile([P, E], f32, tag="gsum")
                    nc.vector.reduce_sum(
                        gsum[:], probs[:].rearrange("p j e -> p e j"), axis=X
                    )
                    nc.vector.tensor_add(acc_sb[:], acc_sb[:], gsum[:])

                    # top-2 on exp values (same order as probs; norm cancels)
                    max84 = spool.tile([P, JT, 8], f32, tag="max84")
                    idx84 = spool.tile([P, JT, 8], mybir.dt.uint32, tag="idx84")
                    for j in range(JT):
                        nc.vector.max(out=max84[:, j, :], in_=ex[:, j, :])
                    for j in range(JT):
                        nc.vector.max_index(
                            idx84[:, j, :], max84[:, j, :], ex[:, j, :]
                        )
                    s2 = spool.tile([P, JT], f32, tag="s2")
                    nc.vector.reduce_sum(s2[:], max84[:, :, 0:TOP_K], axis=X)
                    r2 = spool.tile([P, JT], f32, tag="r2")
                    nc.vector.reciprocal(r2[:], s2[:])
                    sc4 = spool.tile([P, JT, TOP_K], f32, tag="sc4")
                    nc.vector.tensor_mul(
                        sc4[:], max84[:, :, 0:TOP_K],
                        r2[:].to_broadcast([P, JT, TOP_K])
                    )
                    blk = scores.ap()[tb * TB:(tb + 1) * TB, :]
                    nc.sync.dma_start(
                        blk.rearrange("(j p) k -> p j k", p=P), sc4[:]
                    )
                    iblk = idx.ap()[tb * TB:(tb + 1) * TB, :]
                    nc.sync.dma_start(
                        iblk.rearrange("(j p) k -> p j k", p=P),
                        idx84[:, :, 0:TOP_K],
                    )

            nc.sync.dma_start(pacc.ap()[:], acc_sb[:])

    nc.compile()
    return nc


def get_program():
    global _PROGRAM
    if _PROGRAM is None:
        _PROGRAM = _build_program()
    return _PROGRAM


def kernel(x: np.ndarray, W: np.ndarray):
    global LAST_RESULT
    nc = get_program()
    x = np.asarray(x, dtype=np.float32)
    W = np.asarray(W, dtype=np.float32)
    wT = np.ascontiguousarray(W.T)
    in_maps = [
        {
            "xT": np.ascontiguousarray(x[i * TPC:(i + 1) * TPC, :].T),
            "wT": wT,
        }
        for i in range(N_CORES)
    ]
    res = bass_utils.run_bass_kernel_spmd(nc, in_maps, core_ids=list(range(N_CORES)))
    LAST_RESULT = res
    outs = res.results

    topk_scores = np.concatenate([np.asarray(o["scores"]) for o in outs], axis=0)
    topk_indices = (
        np.concatenate([np.asarray(o["idx"]) for o in outs], axis=0)
        .astype(np.uint32)
        .view(np.int32)
    )
    part = np.stack([np.asarray(o["pacc"]) for o in outs], axis=0)  # [8, 128, E]
    expert_prob = part.astype(np.float64).sum(axis=(0, 1)) / N_TOKENS
    loss = np.float32(np.sum(expert_prob * np.log(expert_prob + EPS)))
    return topk_scores, topk_indices, loss


if __name__ == "__main__":
    rng = np.random.default_rng(0)
    x = rng.standard_normal((N_TOKENS, D_MODEL), dtype=np.float32)
    W = rng.standard_normal((NUM_EXPERTS, D_MODEL), dtype=np.float32) / np.sqrt(D_MODEL)
    s, i, l = kernel(x, W)
    print(s.shape, s.dtype, i.shape, i.dtype, l)


# revision 10
# speedup vs baseline: 2.0911x; 1.1462x over previous
"""MoE gate (softmax + top-2 + load-balance loss) on 8 Trainium2 NeuronCores.

Data-parallel: token dim (16384) sharded 8 ways, gate weight replicated.
Per core: logits = x_shard @ W.T via PE (f32), softmax + top-2 via DVE/ACT,
partial per-expert prob sums returned per core; the scalar load-balance loss
is finished on host (tiny reduction).

x arrives [n, d] but the PE contracts along the partition axis, so each
core's shard is fed pre-transposed ([d, n], done on host) — natural-layout
DMA loads, no on-device transpose.

Matmul arrangement: the small gate weight W^T chunk [128, 64] is the
stationary operand (cheap reload), x streams as the moving operand with the
full 512-wide free dim, producing logits transposed [64 experts, 512 tokens]
in PSUM. A PE transpose (identity matmul) flips each 128-token slice back to
[128, 64] for the row-wise softmax/top-k chain.
"""

import os

import numpy as np

import concourse.bacc as bacc
import concourse.mybir as mybir
import concourse.tile as tile
from concourse import bass_utils
from concourse.masks import make_identity

N_TOKENS = 16384
D_MODEL = 4096
NUM_EXPERTS = 64
TOP_K = 2
EPS = 1e-8
N_CORES = 8

TPC = N_TOKENS // N_CORES  # tokens per core (2048)
P = 128                    # partitions
DC = D_MODEL // P          # contraction chunks (32)
TB = 512                   # token block (fp32 moving-free-dim / PSUM bank max)
NB = TPC // TB             # token blocks per core (4)
JT = TB // P               # 128-token tiles per block (4)

_PROGRAM = None
LAST_RESULT = None  # BassKernelResults of the most recent run (for test harness)


def _build_program():
    use_f32r = bool(int(os.environ.get("MOE_F32R", "0")))
    nc = bacc.Bacc(
        "TRN2",
        target_bir_lowering=False,
        debug=False,
        enable_asserts=False,
        num_devices=N_CORES,
    )
    f32 = mybir.dt.float32
    mm_dt = mybir.dt.float32r if use_f32r else f32
    xT = nc.dram_tensor("xT", [D_MODEL, TPC], f32, kind="ExternalInput")
    wS = nc.dram_tensor("wS", [P, DC * NUM_EXPERTS], f32, kind="ExternalInput")
    scores = nc.dram_tensor("scores", [TPC, TOP_K], f32, kind="ExternalOutput")
    idx = nc.dram_tensor("idx", [TPC, TOP_K], mybir.dt.uint32, kind="ExternalOutput")
    pacc = nc.dram_tensor("pacc", [P, NUM_EXPERTS], f32, kind="ExternalOutput")

    E = NUM_EXPERTS
    with tile.TileContext(nc) as tc:
        with (
            tc.tile_pool(name="wp", bufs=1) as wpool,
            tc.tile_pool(name="xp", bufs=6) as xpool,
            tc.tile_pool(name="pp", bufs=2, space="PSUM") as pspool,
            tc.tile_pool(name="sp", bufs=3) as spool,
            tc.tile_pool(name="ap", bufs=1) as apool,
        ):
            # Gate weight, all 32 K-chunks resident: chunk d at wt_sb[:, d, :]
            wt_sb = wpool.tile([P, DC, E], f32)
            nc.sync.dma_start(wt_sb[:], wS.ap().rearrange("p (c e) -> p c e", c=DC))
            ident = wpool.tile([E, E], f32)
            make_identity(nc, ident[:])

            acc_sb = apool.tile([P, E], f32)
            nc.vector.memset(acc_sb[:], 0.0)

            X = mybir.AxisListType.X
            for h in range(2):  # halves of this core's tokens (1024 each)
                # logits^T per 512-token sub-block, two half-sums stacked on
                # the partition axis: [0:64] = even d-chunks, [64:128] = odd.
                # The two col-groups of the PE array run concurrently
                # (tile_position), using all 128 columns despite M=64.
                pT = [
                    pspool.tile([P, TB], f32, tag="pT", name=f"pT{h}_{b}", bufs=4)
                    for b in range(2)
                ]
                for dp in range(DC // 2):
                    xt = xpool.tile([P, 2, 2 * TB], f32, tag="xt",
                                    name=f"xt{h}_{dp}", bufs=12)
                    dma_eng = nc.sync if dp % 2 == 0 else nc.scalar
                    dma_eng.dma_start(
                        xt[:],
                        xT.ap()[2 * dp * P:(2 * dp + 2) * P,
                                h * 2 * TB:(h + 1) * 2 * TB]
                        .rearrange("(c p) n -> p c n", c=2),
                    )
                    for b in range(2):
                        for c in range(2):
                            nc.tensor.matmul(
                                pT[b][c * E:(c + 1) * E, :],
                                wt_sb[:, 2 * dp + c, :].bitcast(mm_dt),
                                xt[:, c, b * TB:(b + 1) * TB].bitcast(mm_dt),
                                start=(dp == 0),
                                stop=(dp == DC // 2 - 1),
                                tile_position=(0, c * E),
                                skip_group_check=True,
                            )
                for b in range(2):
                    tb = 2 * h + b
                    lt = spool.tile([E, TB], f32, tag="lt")
                    nc.vector.tensor_copy(lt[:], pT[b][0:E, :])
                    nc.vector.tensor_add(lt[:], lt[:], pT[b][E:2 * E, :])
                    # [128 tokens, 4 tiles, 64 experts] logits, one PSUM bank
                    p4 = pspool.tile([P, JT, E], f32, tag="p4",
                                     name=f"p4_{tb}", bufs=2)
                    for j in range(JT):
                        nc.tensor.transpose(
                            p4[:, j, :], lt[:, j * P:(j + 1) * P], ident[:]
                        )
                    neg_m = spool.tile([P, JT], f32, tag="negm")
                    nc.vector.reduce_max(neg_m[:], p4[:], axis=X, negate=True)
                    epre = spool.tile([P, JT, E], f32, tag="epre")
                    nc.vector.tensor_add(
                        epre[:], p4[:], neg_m[:].to_broadcast([P, JT, E])
                    )
                    ex = spool.tile([P, JT, E], f32, tag="ex")
                    nc.scalar.activation(
                        ex[:], epre[:], mybir.ActivationFunctionType.Exp
                    )
                    s = spool.tile([P, JT], f32, tag="s")
                    nc.vector.reduce_sum(s[:], ex[:], axis=X)
                    r = spool.tile([P, JT], f32, tag="r")
                    nc.vector.reciprocal(r[:], s[:])
                    probs = spool.tile([P, JT, E], f32, tag="probs")
                    nc.vector.tensor_mul(
                        probs[:], ex[:], r[:].to_broadcast([P, JT, E])
                    )
                    gsum = spool.tile([P, E], f32, tag="gsum")
                    nc.vector.reduce_sum(
                        gsum[:], probs[:].rearrange("p j e -> p e j"), axis=X
                    )
                    nc.vector.tensor_add(acc_sb[:], acc_sb[:], gsum[:])

                    # top-2 on exp values (same order as probs; norm cancels)
                    max84 = spool.tile([P, JT, 8], f32, tag="max84")
                    idx84 = spool.tile([P, JT, 8], mybir.dt.uint32, tag="idx84")
                    for j in range(JT):
                        nc.vector.max(out=max84[:, j, :], in_=ex[:, j, :])
                    for j in range(JT):
                        nc.vector.max_index(
                            idx84[:, j, :], max84[:, j, :], ex[:, j, :]
                        )
                    s2 = spool.tile([P, JT], f32, tag="s2")
                    nc.vector.reduce_sum(s2[:], max84[:, :, 0:TOP_K], axis=X)
                    r2 = spool.tile([P, JT], f32, tag="r2")
                    nc.vector.reciprocal(r2[:], s2[:])
                    sc4 = spool.tile([P, JT, TOP_K], f32, tag="sc4")
                    nc.vector.tensor_mul(
                        sc4[:], max84[:, :, 0:TOP_K],
                        r2[:].to_broadcast([P, JT, TOP_K])
                    )
                    blk = scores.ap()[tb * TB:(tb + 1) * TB, :]
                    nc.sync.dma_start(
                        blk.rearrange("(j p) k -> p j k", p=P), sc4[:]
                    )
                    iblk = idx.ap()[tb * TB:(tb + 1) * TB, :]
                    nc.sync.dma_start(
                        iblk.rearrange("(j p) k -> p j k", p=P),
                        idx84[:, :, 0:TOP_K],
                    )

            nc.sync.dma_start(pacc.ap()[:], acc_sb[:])

    nc.compile()
    return nc


def get_program():
    global _PROGRAM
    if _PROGRAM is None:
        _PROGRAM = _build_program()
    return _PROGRAM


def kernel(x: np.ndarray, W: np.ndarray):
    global LAST_RESULT
    nc = get_program()
    x = np.asarray(x, dtype=np.float32)
    W = np.asarray(W, dtype=np.float32)
    # wS[p, c*E+e] = W[e, c*128+p] — contiguous per-partition weight load
    wS = np.ascontiguousarray(
        W.T.reshape(DC, P, NUM_EXPERTS).transpose(1, 0, 2).reshape(P, DC * NUM_EXPERTS)
    )
    in_maps = [
        {
            "xT": np.ascontiguousarray(x[i * TPC:(i + 1) * TPC, :].T),
            "wS": wS,
        }
        for i in range(N_CORES)
    ]
    res = bass_utils.run_bass_kernel_spmd(nc, in_maps, core_ids=list(range(N_CORES)))
    LAST_RESULT = res
    outs = res.results

    topk_scores = np.concatenate([np.asarray(o["scores"]) for o in outs], axis=0)
    topk_indices = (
        np.concatenate([np.asarray(o["idx"]) for o in outs], axis=0)
        .astype(np.uint32)
        .view(np.int32)
    )
    part = np.stack([np.asarray(o["pacc"]) for o in outs], axis=0)  # [8, 128, E]
    expert_prob = part.astype(np.float64).sum(axis=(0, 1)) / N_TOKENS
    loss = np.float32(np.sum(expert_prob * np.log(expert_prob + EPS)))
    return topk_scores, topk_indices, loss


if __name__ == "__main__":
    rng = np.random.default_rng(0)
    x = rng.standard_normal((N_TOKENS, D_MODEL), dtype=np.float32)
    W = rng.standard_normal((NUM_EXPERTS, D_MODEL), dtype=np.float32) / np.sqrt(D_MODEL)
    s, i, l = kernel(x, W)
    print(s.shape, s.dtype, i.shape, i.dtype, l)
